# revision 7
# baseline (speedup 1.0000x reference)
"""Trainium2 Bass kernel for nn_BiModalF (bi-modal attention pooling network).

Strategy
--------
Data-parallel over batch B=8 across the 8 NeuronCores (one batch element per
core).  Inside each core the key algebraic reformulation is that the reference's

    att = relu(ca[:,:,None,:] + pa[:,None,:,:]) @ Wi.T + bi          # [C,P,H]
    compound_weights = sigmoid(mean_p att)                            # [C,H]
    protein_weights  = sigmoid(mean_c att)                            # [P,H]

commutes the (linear) Wi projection with the (linear) means, so the giant
[C,P,H] tensor never needs a matmul:

    Sc[c,h] = sum_p relu(ca[c,h] + pa[p,h])      -> compound_weights
    Sp[p,h] = sum_c relu(ca[c,h] + pa[p,h])      -> protein_weights

Everything is laid out transposed ([H, *] with H=128 on partitions) so the
broadcast add of ca[c, :] is a per-partition scalar op:

  - ACT path:  R_c = relu(paT + ca_c)  with accum_out -> Sc column (free-dim sum)
  - DVE path:  R_c = max(paT, -ca_c)   (= relu - ca_c), accum_out -> sum
               (corrected afterwards: Sc += P*ca_c, Sp += sum_dve ca_c)
  - PE path:   Sp accumulated in PSUM via identity matmuls over the R_c tiles.

R tiles are fp16 (PE streams 16-bit at 1 col/cycle, DVE gets 4x mode);
projections run as float32r matmuls (~1.5e-4 rel err); reductions accumulate
in fp32.
"""

import numpy as np

B, C, P = 8, 128, 1024
PROT_DIM, ATOM_DIM, H = 1024, 34, 128
NCORES = 8
W = 512  # matmul moving-operand window

# c's handled by the scalar engine (every 4th); rest by the vector engine
ACT_EVERY = 4

_CACHE = {}


def _build():
    import concourse.bacc as bacc
    import concourse.tile as tile
    from concourse import mybir
    from concourse.masks import make_identity
    from contextlib import ExitStack

    F32 = mybir.dt.float32
    F32R = mybir.dt.float32r
    F16 = mybir.dt.float16
    AF = mybir.ActivationFunctionType
    ALU = mybir.AluOpType

    nc = bacc.Bacc("TRN2", target_bir_lowering=False, debug=False,
                   num_devices=NCORES)

    # ---- I/O ----
    prot_d = nc.dram_tensor("protein", [P, PROT_DIM], F32, kind="ExternalInput").ap()
    comp_d = nc.dram_tensor("compound", [C, ATOM_DIM], F32, kind="ExternalInput").ap()
    Wp_d = nc.dram_tensor("Wp", [H, PROT_DIM], F32, kind="ExternalInput").ap()
    bp_d = nc.dram_tensor("bp", [H], F32, kind="ExternalInput").ap()
    Wc_d = nc.dram_tensor("Wc", [H, ATOM_DIM], F32, kind="ExternalInput").ap()
    bc_d = nc.dram_tensor("bc", [H], F32, kind="ExternalInput").ap()
    Wpa_d = nc.dram_tensor("Wpa", [H, H], F32, kind="ExternalInput").ap()
    bpa_d = nc.dram_tensor("bpa", [H], F32, kind="ExternalInput").ap()
    Wca_d = nc.dram_tensor("Wca", [H, H], F32, kind="ExternalInput").ap()
    bca_d = nc.dram_tensor("bca", [H], F32, kind="ExternalInput").ap()
    Wi_d = nc.dram_tensor("Wi", [H, H], F32, kind="ExternalInput").ap()
    bi_d = nc.dram_tensor("bi", [H], F32, kind="ExternalInput").ap()
    W1_d = nc.dram_tensor("W1", [1024, 2 * H], F32, kind="ExternalInput").ap()
    b1_d = nc.dram_tensor("b1", [1024], F32, kind="ExternalInput").ap()
    W2_d = nc.dram_tensor("W2", [512, 1024], F32, kind="ExternalInput").ap()
    b2_d = nc.dram_tensor("b2", [512], F32, kind="ExternalInput").ap()
    Wo_d = nc.dram_tensor("Wo", [2, 512], F32, kind="ExternalInput").ap()
    bo_d = nc.dram_tensor("bo", [2], F32, kind="ExternalInput").ap()
    out_d = nc.dram_tensor("out", [2, 1], F32, kind="ExternalOutput").ap()

    with tile.TileContext(nc) as tc:
        with ExitStack() as ctx:
            one = ctx.enter_context(tc.tile_pool(name="one", bufs=1))
            natp = ctx.enter_context(tc.tile_pool(name="natp", bufs=1))
            ptp = ctx.enter_context(tc.tile_pool(name="ptp", bufs=2))
            r_act = ctx.enter_context(tc.tile_pool(name="r_act", bufs=2))
            r_dve = ctx.enter_context(tc.tile_pool(name="r_dve", bufs=3))
            ps_tp = ctx.enter_context(tc.tile_pool(name="ps_tp", bufs=3, space="PSUM"))
            ps_mm = ctx.enter_context(tc.tile_pool(name="ps_mm", bufs=1, space="PSUM"))
            ps_sp = ctx.enter_context(tc.tile_pool(name="ps_sp", bufs=1, space="PSUM"))
            ps_sm = ctx.enter_context(tc.tile_pool(name="ps_sm", bufs=1, space="PSUM"))

            # ---------------- DMA loads ----------------
            prot_nat = []
            for pi in range(8):
                t = natp.tile([128, PROT_DIM], F32, tag=f"prot{pi}")
                nc.sync.dma_start(t, prot_d[pi * 128:(pi + 1) * 128, :])
                prot_nat.append(t)
            Wp_nat = one.tile([H, PROT_DIM], F32)
            nc.sync.dma_start(Wp_nat, Wp_d)
            comp_nat = one.tile([C, ATOM_DIM], F32)
            nc.sync.dma_start(comp_nat, comp_d)
            Wc_nat = one.tile([H, ATOM_DIM], F32)
            nc.sync.dma_start(Wc_nat, Wc_d)
            Wpa_nat = one.tile([H, H], F32)
            nc.sync.dma_start(Wpa_nat, Wpa_d)
            Wca_nat = one.tile([H, H], F32)
            nc.sync.dma_start(Wca_nat, Wca_d)
            Wi_nat = one.tile([H, H], F32)
            nc.sync.dma_start(Wi_nat, Wi_d)
            W1_nat = one.tile([128, 8, 256], F32)
            nc.sync.dma_start(W1_nat, W1_d.rearrange("(mi p) k -> p mi k", p=128))
            W2_nat = one.tile([128, 4, 1024], F32)
            nc.sync.dma_start(W2_nat, W2_d.rearrange("(mi p) k -> p mi k", p=128))
            WoT = one.tile([128, 4, 2], F32)
            for o in range(2):
                nc.sync.dma_start(WoT[:, :, o],
                                  Wo_d[o, :].rearrange("(c p) -> p c", p=128))

            def load_bias(ap, n, name):
                t = one.tile([n, 1], F32, tag=f"bias_{name}")
                nc.sync.dma_start(t, ap.rearrange("(h o) -> h o", o=1))
                return t

            bp_col = load_bias(bp_d, H, "bp")
            bc_col = load_bias(bc_d, H, "bc")
            bpa_col = load_bias(bpa_d, H, "bpa")
            bca_col = load_bias(bca_d, H, "bca")
            bi_col = load_bias(bi_d, H, "bi")
            b1_col = one.tile([128, 8], F32)
            nc.sync.dma_start(b1_col, b1_d.rearrange("(mi p) -> p mi", p=128))
            b2_col = one.tile([128, 4], F32)
            nc.sync.dma_start(b2_col, b2_d.rearrange("(mi p) -> p mi", p=128))
            bo_col = load_bias(bo_d, 2, "bo")

            ident_f = one.tile([128, 128], F32)
            make_identity(nc, ident_f)
            ident_h = one.tile([128, 128], F16)
            make_identity(nc, ident_h)

            # helper: PE transpose of a [128,128] f32 tile -> SBUF tile (dtype dt)
            cp_toggle = [0]

            def transpose_to(dst_ap, src_ap, n_out=128):
                pst = ps_tp.tile([128, 128], F32, tag="tp")
                nc.tensor.transpose(pst[:n_out, :], src_ap, ident_f)
                if cp_toggle[0] % 2 == 0:
                    nc.scalar.copy(dst_ap, pst[:n_out, :])
                else:
                    nc.vector.tensor_copy(dst_ap, pst[:n_out, :])
                cp_toggle[0] += 1

            # ---------------- compound side (tiny) ----------------
            # compoundT [34, 128], WcT [34, 128]
            compT = one.tile([ATOM_DIM, C], F32)
            transpose_to(compT, comp_nat, n_out=ATOM_DIM)
            WcT = one.tile([ATOM_DIM, H], F32)
            transpose_to(WcT, Wc_nat, n_out=ATOM_DIM)

            # cpT = Wc @ compoundT + bc   [h, c]
            ps_cp = ps_sm.tile([H, C], F32, tag="small")
            nc.tensor.matmul(ps_cp, WcT, compT, start=True, stop=True)
            cpT = one.tile([H, C], F32)
            nc.scalar.activation(out=cpT, in_=ps_cp, func=AF.Identity,
                                 bias=bc_col, scale=1.0)
            cpT_r = one.tile([H, C], F32R)
            nc.vector.tensor_copy(cpT_r, cpT)

            # caT = Wca @ cpT + bca   [h, c]
            WcaT = one.tile([H, H], F32R)
            transpose_to(WcaT, Wca_nat)
            ps_ca = ps_sm.tile([H, C], F32, tag="small")
            nc.tensor.matmul(ps_ca, WcaT, cpT_r, start=True, stop=True)
            caT = one.tile([H, C], F32)
            nc.scalar.activation(out=caT, in_=ps_ca, func=AF.Identity,
                                 bias=bca_col, scale=1.0)
            neg_caT = one.tile([H, C], F32)
            nc.vector.tensor_scalar(out=neg_caT, in0=caT, scalar1=-1.0,
                                    scalar2=None, op0=ALU.mult)

            # ---------------- protein projections ----------------
            # WpT [k, h] chunks
            WpT = one.tile([128, 8, H], F32R)
            for kj in range(8):
                transpose_to(WpT[:, kj, :], Wp_nat[:, kj * 128:(kj + 1) * 128])

            WpaT = one.tile([H, H], F32R)
            transpose_to(WpaT, Wpa_nat)
            WiT_r = one.tile([H, H], F32R)
            transpose_to(WiT_r, Wi_nat)
            WiT_f = one.tile([H, H], F32)
            nc.vector.tensor_copy(WiT_f, WiT_r.bitcast(F32))

            # proteinT chunks + ppT matmuls, window-pipelined
            ps_pp = ps_mm.tile([H, P], F32, tag="mmbig")
            for kj in range(8):
                pt = ptp.tile([128, P], F32R, tag="protT")
                for pi in range(8):
                    transpose_to(pt[:, pi * 128:(pi + 1) * 128],
                                 prot_nat[pi][:, kj * 128:(kj + 1) * 128])
                for w in range(2):
                    nc.tensor.matmul(ps_pp[:, w * W:(w + 1) * W],
                                     WpT[:, kj, :],
                                     pt[:, w * W:(w + 1) * W],
                                     start=(kj == 0), stop=(kj == 7))
            # ppT = psum + bp  (f32r for pa matmul; bitcast-f32 view for the pool stage)
            ppT = one.tile([H, P], F32R)
            for w in range(2):
                nc.scalar.activation(out=ppT[:, w * W:(w + 1) * W],
                                     in_=ps_pp[:, w * W:(w + 1) * W],
                                     func=AF.Identity, bias=bp_col, scale=1.0)

            # paT = Wpa @ ppT + bpa  -> fp16
            ps_pa = ps_mm.tile([H, P], F32, tag="mmbig")
            for w in range(2):
                nc.tensor.matmul(ps_pa[:, w * W:(w + 1) * W], WpaT,
                                 ppT[:, w * W:(w + 1) * W], start=True, stop=True)
            paT = one.tile([H, P], F16)
            for w in range(2):
                nc.scalar.activation(out=paT[:, w * W:(w + 1) * W],
                                     in_=ps_pa[:, w * W:(w + 1) * W],
                                     func=AF.Identity, bias=bpa_col, scale=1.0)

            # ---------------- the c-loop ----------------
            n_act = C // ACT_EVERY
            n_dve = C - n_act
            Sc_a = one.tile([H, n_act], F32)
            Sc_d = one.tile([H, n_dve], F32)
            ps_Sp = ps_sp.tile([H, P], F32)

            for c in range(C):
                is_act = (c % ACT_EVERY == ACT_EVERY - 1)
                if is_act:
                    r = r_act.tile([H, P], F16, tag="ra")
                    nc.scalar.activation(
                        out=r, in_=paT, func=AF.Relu,
                        bias=caT[:, c:c + 1], scale=1.0,
                        accum_out=Sc_a[:, c // ACT_EVERY:c // ACT_EVERY + 1])
                else:
                    r = r_dve.tile([H, P], F16, tag="rd")
                    di = c - c // ACT_EVERY
                    nc.vector.tensor_scalar(
                        out=r, in0=paT,
                        scalar1=neg_caT[:, c:c + 1], scalar2=None,
                        op0=ALU.max, op1=ALU.add,
                        accum_out=Sc_d[:, di:di + 1])
                for w in range(2):
                    nc.tensor.matmul(ps_Sp[:, w * W:(w + 1) * W], ident_h,
                                     r[:, w * W:(w + 1) * W],
                                     start=(c == 0), stop=(c == C - 1))

            # ---------------- corrections ----------------
            # strided view of caT's DVE columns: [h, g, f] with f = first 3 of 4
            caT_dve = caT.rearrange("h (g f) -> h g f", f=ACT_EVERY)[:, :, 0:ACT_EVERY - 1]
            Sc_d_v = Sc_d.rearrange("h (g f) -> h g f", f=ACT_EVERY - 1)
            # Sc_d += P * caT_dve
            tmp_corr = one.tile([H, n_dve], F32)
            tmp_corr_v = tmp_corr.rearrange("h (g f) -> h g f", f=ACT_EVERY - 1)
            nc.vector.tensor_scalar(out=tmp_corr_v, in0=caT_dve, scalar1=float(P),
                                    scalar2=None, op0=ALU.mult)
            nc.vector.tensor_add(Sc_d, Sc_d, tmp_corr)
            # Sp bias correction: sum of DVE ca columns
            sum_dve_ca = one.tile([H, 1], F32)
            nc.vector.tensor_reduce(sum_dve_ca, caT_dve, axis=mybir.AxisListType.XY,
                                    op=ALU.add)

            Sp_sb = one.tile([H, P], F32R)
            nc.scalar.activation(out=Sp_sb, in_=ps_Sp, func=AF.Identity,
                                 bias=sum_dve_ca, scale=1.0)

            # ---------------- attention weights + pools ----------------
            # compound: cw = sigmoid((Wi @ ScT)/P + bi)   [h, c]
            ps_cw = ps_sm.tile([H, C], F32, tag="small")
            nc.tensor.matmul(ps_cw[:, 0:n_act], WiT_f, Sc_a, start=True, stop=True)
            nc.tensor.matmul(ps_cw[:, n_act:C], WiT_f, Sc_d, start=True, stop=True)
            # write sigmoid outputs back into natural column order
            cw1 = one.tile([H, C], F32)
            cw1_act_v = cw1.rearrange("h (g f) -> h g f", f=ACT_EVERY)[:, :, ACT_EVERY - 1:ACT_EVERY]
            cw1_dve_v = cw1.rearrange("h (g f) -> h g f", f=ACT_EVERY)[:, :, 0:ACT_EVERY - 1]
            ps_cw_act_v = ps_cw[:, 0:n_act].rearrange("h (g f) -> h g f", f=1)
            ps_cw_dve_v = ps_cw[:, n_act:C].rearrange("h (g f) -> h g f", f=ACT_EVERY - 1)
            nc.scalar.activation(out=cw1_act_v, in_=ps_cw_act_v, func=AF.Sigmoid,
                                 bias=bi_col, scale=1.0 / P)
            nc.scalar.activation(out=cw1_dve_v, in_=ps_cw_dve_v, func=AF.Sigmoid,
                                 bias=bi_col, scale=1.0 / P)
            nc.vector.tensor_scalar(out=cw1, in0=cw1, scalar1=0.5, scalar2=None,
                                    op0=ALU.add)
            # weighted compound pool -> pair[:,0]
            pair = one.tile([H, 2], F32)
            junk_c = one.tile([H, C], F32)
            nc.vector.tensor_mul(junk_c, cpT, cw1)
            nc.vector.tensor_reduce(pair[:, 0:1], junk_c,
                                    axis=mybir.AxisListType.X, op=ALU.max)

            # protein: pw = sigmoid((Wi @ SpT)/C + bi)  [h, p]
            ps_pw = ps_mm.tile([H, P], F32, tag="mmbig")
            for w in range(2):
                nc.tensor.matmul(ps_pw[:, w * W:(w + 1) * W], WiT_r,
                                 Sp_sb[:, w * W:(w + 1) * W], start=True, stop=True)
            pw1 = one.tile([H, P], F32)
            nc.scalar.activation(out=pw1, in_=ps_pw, func=AF.Sigmoid,
                                 bias=bi_col, scale=1.0 / C)
            nc.vector.tensor_scalar(out=pw1, in0=pw1, scalar1=0.5, scalar2=None,
                                    op0=ALU.add)
            junk_p = one.tile([H, P], F32)
            nc.vector.tensor_mul(junk_p, ppT.bitcast(F32), pw1)
            nc.vector.tensor_reduce(pair[:, 1:2], junk_p,
                                    axis=mybir.AxisListType.X, op=ALU.max)

            # ---------------- final MLP ----------------
            # W1T/W2T tiles
            W1T = one.tile([128, 2, 8, 128], F32)
            for mi in range(8):
                for kj in range(2):
                    transpose_to(W1T[:, kj, mi, :],
                                 W1_nat[:, mi, kj * 128:(kj + 1) * 128])
            W2T = one.tile([128, 8, 4, 128], F32)
            for mi in range(4):
                for kj in range(8):
                    transpose_to(W2T[:, kj, mi, :],
                                 W2_nat[:, mi, kj * 128:(kj + 1) * 128])

            # x1 = lrelu(W1 @ pair + b1)  as [128, 8] columns
            ps_x1 = ps_sm.tile([128, 8], F32, tag="small")
            for mi in range(8):
                for kj in range(2):
                    nc.tensor.matmul(ps_x1[:, mi:mi + 1], W1T[:, kj, mi, :],
                                     pair[:, kj:kj + 1],
                                     start=(kj == 0), stop=(kj == 1))
            x1 = one.tile([128, 8], F32)
            for mi in range(8):
                nc.scalar.activation(out=x1[:, mi:mi + 1], in_=ps_x1[:, mi:mi + 1],
                                     func=AF.Identity, bias=b1_col[:, mi:mi + 1],
                                     scale=1.0)
            x1s = one.tile([128, 8], F32)
            nc.vector.tensor_scalar(out=x1s, in0=x1, scalar1=0.01, scalar2=None,
                                    op0=ALU.mult)
            nc.vector.tensor_tensor(out=x1, in0=x1, in1=x1s, op=ALU.max)

            # x2 = lrelu(W2 @ x1 + b2)  as [128, 4]
            ps_x2 = ps_sm.tile([128, 4], F32, tag="small")
            for mi in range(4):
                for kj in range(8):
                    nc.tensor.matmul(ps_x2[:, mi:mi + 1], W2T[:, kj, mi, :],
                                     x1[:, kj:kj + 1],
                                     start=(kj == 0), stop=(kj == 7))
            x2 = one.tile([128, 4], F32)
            for mi in range(4):
                nc.scalar.activation(out=x2[:, mi:mi + 1], in_=ps_x2[:, mi:mi + 1],
                                     func=AF.Identity, bias=b2_col[:, mi:mi + 1],
                                     scale=1.0)
            x2s = one.tile([128, 4], F32)
            nc.vector.tensor_scalar(out=x2s, in0=x2, scalar1=0.01, scalar2=None,
                                    op0=ALU.mult)
            nc.vector.tensor_tensor(out=x2, in0=x2, in1=x2s, op=ALU.max)

            # out = Wo @ x2 + bo   [2, 1]
            ps_o = ps_sm.tile([2, 1], F32, tag="small")
            for kj in range(4):
                nc.tensor.matmul(ps_o, WoT[:, kj, :], x2[:, kj:kj + 1],
                                 start=(kj == 0), stop=(kj == 3))
            out_sb = one.tile([2, 1], F32)
            nc.scalar.activation(out=out_sb, in_=ps_o, func=AF.Identity,
                                 bias=bo_col, scale=1.0)
            nc.sync.dma_start(out_d, out_sb)

    nc.compile()
    return nc


def kernel(**inputs):
    from concourse.bass_utils import run_bass_kernel_spmd

    nc = _CACHE.get("nc")
    if nc is None:
        nc = _build()
        _CACHE["nc"] = nc

    weight_names = ["Wp", "bp", "Wc", "bc", "Wpa", "bpa", "Wca", "bca",
                    "Wi", "bi", "W1", "b1", "W2", "b2", "Wo", "bo"]
    in_maps = []
    for b in range(B):
        m = {"protein": np.ascontiguousarray(inputs["protein"][b], dtype=np.float32),
             "compound": np.ascontiguousarray(inputs["compound"][b], dtype=np.float32)}
        for k in weight_names:
            m[k] = np.ascontiguousarray(inputs[k], dtype=np.float32)
        in_maps.append(m)

    res = run_bass_kernel_spmd(nc, in_maps, core_ids=list(range(NCORES)))
    out = np.stack([res.results[b]["out"].reshape(2) for b in range(B)])
    return out.astype(np.float32)


# revision 13
# speedup vs baseline: 1.4145x; 1.4145x over previous
"""Trainium2 Bass kernel for nn_BiModalF (bi-modal attention pooling network).

Data-parallel over batch B=8 across 8 NeuronCores.  Key reformulation: the
Wi projection commutes with the means over the big [C,P,H] relu tensor, so
only two reductions of relu(ca+pa) are needed (Sc over p, Sp over c).
Layout is transposed ([H on partitions]) so the ca broadcast is a
per-partition scalar:
  odd c  -> ACT: relu + accum_out (Sc column)
  even c -> DVE: max(paT,-ca) + accum (corrected later; 2-op-with-accum HW
            semantics apply op0 only to out, reduce with op1)
  PE     -> Sp accumulated via fp16 identity matmuls in PSUM.
Projections in float32r; PE transposes packed 4-per-PSUM-bank; MLP in row
form so LDWEIGHTS is trivial.
"""

import numpy as np

B, C, P = 8, 128, 1024
PROT_DIM, ATOM_DIM, H = 1024, 34, 128
NCORES = 8
W = 512

_CACHE = {}


def _build():
    import concourse.bacc as bacc
    import concourse.tile as tile
    from concourse import mybir
    from concourse.masks import make_identity
    from contextlib import ExitStack

    F32 = mybir.dt.float32
    F32R = mybir.dt.float32r
    F16 = mybir.dt.float16
    AF = mybir.ActivationFunctionType
    ALU = mybir.AluOpType

    nc = bacc.Bacc("TRN2", target_bir_lowering=False, debug=False,
                   num_devices=NCORES)

    prot_d = nc.dram_tensor("protein", [P, PROT_DIM], F32, kind="ExternalInput").ap()
    comp_d = nc.dram_tensor("compound", [C, ATOM_DIM], F32, kind="ExternalInput").ap()
    Wp_d = nc.dram_tensor("Wp", [H, PROT_DIM], F32, kind="ExternalInput").ap()
    bp_d = nc.dram_tensor("bp", [H], F32, kind="ExternalInput").ap()
    Wc_d = nc.dram_tensor("Wc", [H, ATOM_DIM], F32, kind="ExternalInput").ap()
    bc_d = nc.dram_tensor("bc", [H], F32, kind="ExternalInput").ap()
    Wpa_d = nc.dram_tensor("Wpa", [H, H], F32, kind="ExternalInput").ap()
    bpa_d = nc.dram_tensor("bpa", [H], F32, kind="ExternalInput").ap()
    Wca_d = nc.dram_tensor("Wca", [H, H], F32, kind="ExternalInput").ap()
    bca_d = nc.dram_tensor("bca", [H], F32, kind="ExternalInput").ap()
    Wi_d = nc.dram_tensor("Wi", [H, H], F32, kind="ExternalInput").ap()
    bi_d = nc.dram_tensor("bi", [H], F32, kind="ExternalInput").ap()
    W1_d = nc.dram_tensor("W1", [1024, 2 * H], F32, kind="ExternalInput").ap()
    b1_d = nc.dram_tensor("b1", [1024], F32, kind="ExternalInput").ap()
    W2_d = nc.dram_tensor("W2", [512, 1024], F32, kind="ExternalInput").ap()
    b2_d = nc.dram_tensor("b2", [512], F32, kind="ExternalInput").ap()
    Wo_d = nc.dram_tensor("Wo", [2, 512], F32, kind="ExternalInput").ap()
    bo_d = nc.dram_tensor("bo", [2], F32, kind="ExternalInput").ap()
    out_d = nc.dram_tensor("out", [2, 1], F32, kind="ExternalOutput").ap()

    with tile.TileContext(nc) as tc:
        with ExitStack() as ctx:
            one = ctx.enter_context(tc.tile_pool(name="one", bufs=1))
            natp = ctx.enter_context(tc.tile_pool(name="natp", bufs=1))
            ptp = ctx.enter_context(tc.tile_pool(name="ptp", bufs=2))
            r_act = ctx.enter_context(tc.tile_pool(name="r_act", bufs=3))
            r_dve = ctx.enter_context(tc.tile_pool(name="r_dve", bufs=3))
            ps_tp = ctx.enter_context(tc.tile_pool(name="ps_tp", bufs=3, space="PSUM"))
            ps_mm = ctx.enter_context(tc.tile_pool(name="ps_mm", bufs=1, space="PSUM"))
            ps_sp = ctx.enter_context(tc.tile_pool(name="ps_sp", bufs=1, space="PSUM"))
            ps_sm = ctx.enter_context(tc.tile_pool(name="ps_sm", bufs=1, space="PSUM"))

            prot_nat = []
            for pi in range(8):
                t = natp.tile([128, PROT_DIM], F32, tag=f"prot{pi}")
                nc.sync.dma_start(t, prot_d[pi * 128:(pi + 1) * 128, :])
                prot_nat.append(t)
            Wp_nat = one.tile([H, PROT_DIM], F32)
            nc.sync.dma_start(Wp_nat, Wp_d)
            comp_nat = one.tile([C, ATOM_DIM], F32)
            nc.sync.dma_start(comp_nat, comp_d)
            Wc_nat = one.tile([H, ATOM_DIM], F32)
            nc.sync.dma_start(Wc_nat, Wc_d)
            Wpa_nat = one.tile([H, H], F32)
            nc.sync.dma_start(Wpa_nat, Wpa_d)
            Wca_nat = one.tile([H, H], F32)
            nc.sync.dma_start(Wca_nat, Wca_d)
            Wi_nat = one.tile([H, H], F32)
            nc.sync.dma_start(Wi_nat, Wi_d)
            W1_nat = one.tile([128, 8, 256], F32)
            nc.sync.dma_start(W1_nat,
                              W1_d.rearrange("(mi p) k -> p mi k", p=128))
            W2_nat = one.tile([128, 4, 1024], F32)
            nc.sync.dma_start(W2_nat,
                              W2_d.rearrange("(mi p) k -> p mi k", p=128))
            WoT = one.tile([128, 4, 2], F32)
            for o in range(2):
                nc.sync.dma_start(WoT[:, :, o],
                                  Wo_d[o, :].rearrange("(c p) -> p c", p=128))

            def load_bias(ap, n, name):
                t = one.tile([n, 1], F32, tag=f"bias_{name}")
                nc.sync.dma_start(t, ap.rearrange("(h o) -> h o", o=1))
                return t

            bp_col = load_bias(bp_d, H, "bp")
            bc_col = load_bias(bc_d, H, "bc")
            bpa_col = load_bias(bpa_d, H, "bpa")
            bca_col = load_bias(bca_d, H, "bca")
            bi_col = load_bias(bi_d, H, "bi")
            b1_col = one.tile([128, 8], F32)
            nc.sync.dma_start(b1_col, b1_d.rearrange("(mi p) -> p mi", p=128))
            b2_col = one.tile([128, 4], F32)
            nc.sync.dma_start(b2_col, b2_d.rearrange("(mi p) -> p mi", p=128))
            bo_col = load_bias(bo_d, 2, "bo")

            ident_f = one.tile([128, 128], F32)
            make_identity(nc, ident_f)

            ident_h = one.tile([128, 128], F16)
            make_identity(nc, ident_h)

            cp_toggle = [0]

            def transpose_pack(dst_ap, srcs, dt):
                n = len(srcs)
                pst = ps_tp.tile([128, 512], F32, tag="tp")
                for i, s in enumerate(srcs):
                    nc.tensor.matmul(pst[:, i * 128:(i + 1) * 128], s,
                                     ident_f,
                                     is_transpose=True,
                                     start=(i == 0), stop=(i == n - 1),
                                     skip_group_check=True)
                if cp_toggle[0] % 2 == 0:
                    nc.scalar.copy(dst_ap, pst[:, 0:n * 128])
                else:
                    nc.vector.tensor_copy(dst_ap, pst[:, 0:n * 128])
                cp_toggle[0] += 1

            def transpose_one(dst_ap, src_ap, dt, n_out=128):
                pst = ps_tp.tile([128, 512], F32, tag="tp")
                nc.tensor.matmul(pst[:n_out, 0:128], src_ap,
                                 ident_f,
                                 is_transpose=True, start=True, stop=True)
                if cp_toggle[0] % 2 == 0:
                    nc.scalar.copy(dst_ap, pst[:n_out, 0:128])
                else:
                    nc.vector.tensor_copy(dst_ap, pst[:n_out, 0:128])
                cp_toggle[0] += 1

            # compound side
            compT = one.tile([ATOM_DIM, C], F32)
            transpose_one(compT, comp_nat, F32, n_out=ATOM_DIM)
            WcT = one.tile([ATOM_DIM, H], F32)
            transpose_one(WcT, Wc_nat, F32, n_out=ATOM_DIM)

            ps_cp = ps_sm.tile([H, C], F32, tag="small")
            nc.tensor.matmul(ps_cp, WcT, compT, start=True, stop=True)
            cpT = one.tile([H, C], F32)
            nc.scalar.activation(out=cpT, in_=ps_cp, func=AF.Identity,
                                 bias=bc_col, scale=1.0)
            cpT_r = one.tile([H, C], F32R)
            nc.vector.tensor_copy(cpT_r, cpT)

            WcaT = one.tile([H, H], F32R)
            transpose_one(WcaT, Wca_nat, F32R)
            ps_ca = ps_sm.tile([H, C], F32, tag="small")
            nc.tensor.matmul(ps_ca, WcaT, cpT_r, start=True, stop=True)
            caT = one.tile([H, C], F32)
            nc.scalar.activation(out=caT, in_=ps_ca, func=AF.Identity,
                                 bias=bca_col, scale=1.0)
            neg_caT = one.tile([H, C], F32)
            nc.vector.tensor_scalar(out=neg_caT, in0=caT, scalar1=-1.0,
                                    scalar2=None, op0=ALU.mult)

            # protein projections
            WpT = one.tile([128, 8, H], F32R)
            for g in range(2):
                transpose_pack(WpT[:, 4 * g:4 * g + 4, :],
                               [Wp_nat[:, (4 * g + j) * 128:(4 * g + j + 1) * 128]
                                for j in range(4)], F32R)

            WpaT = one.tile([H, H], F32R)
            transpose_one(WpaT, Wpa_nat, F32R)
            WiT_r = one.tile([H, H], F32R)
            transpose_one(WiT_r, Wi_nat, F32R)
            WiT_f = one.tile([H, H], F32)
            nc.vector.tensor_copy(WiT_f, WiT_r.bitcast(F32))

            ps_pp = ps_mm.tile([H, P], F32, tag="mmbig")
            for kj in range(8):
                pt = ptp.tile([128, P], F32R, tag="protT")
                for g in range(2):
                    transpose_pack(pt[:, g * 512:(g + 1) * 512],
                                   [prot_nat[4 * g + j][:, kj * 128:(kj + 1) * 128]
                                    for j in range(4)], F32R)
                for w in range(2):
                    nc.tensor.matmul(ps_pp[:, w * W:(w + 1) * W],
                                     WpT[:, kj, :],
                                     pt[:, w * W:(w + 1) * W],
                                     start=(kj == 0), stop=(kj == 7))
            ppT = one.tile([H, P], F32R)
            for w in range(2):
                nc.scalar.activation(out=ppT[:, w * W:(w + 1) * W],
                                     in_=ps_pp[:, w * W:(w + 1) * W],
                                     func=AF.Identity, bias=bp_col, scale=1.0)

            ps_pa = ps_mm.tile([H, P], F32, tag="mmbig")
            for w in range(2):
                nc.tensor.matmul(ps_pa[:, w * W:(w + 1) * W], WpaT,
                                 ppT[:, w * W:(w + 1) * W], start=True, stop=True)
            paT = one.tile([H, P], F16)
            for w in range(2):
                nc.scalar.activation(out=paT[:, w * W:(w + 1) * W],
                                     in_=ps_pa[:, w * W:(w + 1) * W],
                                     func=AF.Identity, bias=bpa_col, scale=1.0)

            # c-loop: odd c -> ACT, even c -> DVE
            Sc_a = one.tile([H, C // 2], F32)
            Sc_d = one.tile([H, C // 2], F32)
            ps_Sp = ps_sp.tile([H, P], F32)

            for c in range(C):
                if c % 2 == 1:
                    r = r_act.tile([H, P], F16, tag="ra")
                    nc.scalar.activation(
                        out=r, in_=paT, func=AF.Relu,
                        bias=caT[:, c:c + 1], scale=1.0,
                        accum_out=Sc_a[:, c // 2:c // 2 + 1])
                else:
                    r = r_dve.tile([H, P], F16, tag="rd")
                    nc.vector.tensor_scalar(
                        out=r, in0=paT,
                        scalar1=neg_caT[:, c:c + 1], scalar2=None,
                        op0=ALU.max, op1=ALU.add,
                        accum_out=Sc_d[:, c // 2:c // 2 + 1])
                for w in range(2):
                    nc.tensor.matmul(ps_Sp[:, w * W:(w + 1) * W], ident_h,
                                     r[:, w * W:(w + 1) * W],
                                     start=(c == 0), stop=(c == C - 1))

            # corrections (even/DVE columns used the max-shift form)
            caT_even = caT.rearrange("h (g f) -> h g f", f=2)[:, :, 0:1]
            nc.vector.scalar_tensor_tensor(
                out=Sc_d, in0=caT_even, scalar=float(P), in1=Sc_d,
                op0=ALU.mult, op1=ALU.add)
            sum_dve_ca = one.tile([H, 1], F32)
            nc.vector.tensor_reduce(sum_dve_ca, caT_even, axis=mybir.AxisListType.XY,
                                    op=ALU.add)
            Sp_sb = one.tile([H, P], F32R)
            nc.scalar.activation(out=Sp_sb, in_=ps_Sp, func=AF.Identity,
                                 bias=sum_dve_ca, scale=1.0)

            # attention weights + pools
            nact = C // 2
            ps_cw = ps_sm.tile([H, C], F32, tag="small")
            nc.tensor.matmul(ps_cw[:, 0:nact], WiT_f, Sc_a, start=True, stop=True)
            nc.tensor.matmul(ps_cw[:, nact:C], WiT_f, Sc_d, start=True, stop=True)
            cw1 = one.tile([H, C], F32)
            cw1_odd = cw1.rearrange("h (g f) -> h g f", f=2)[:, :, 1:2]
            cw1_even = cw1.rearrange("h (g f) -> h g f", f=2)[:, :, 0:1]
            nc.scalar.activation(out=cw1_odd,
                                 in_=ps_cw[:, 0:nact].rearrange("h (g f) -> h g f", f=1),
                                 func=AF.Sigmoid, bias=bi_col, scale=1.0 / P)
            nc.scalar.activation(out=cw1_even,
                                 in_=ps_cw[:, nact:C].rearrange("h (g f) -> h g f", f=1),
                                 func=AF.Sigmoid, bias=bi_col, scale=1.0 / P)
            nc.vector.tensor_scalar(out=cw1, in0=cw1, scalar1=0.5, scalar2=None,
                                    op0=ALU.add)
            pair = one.tile([H, 2], F32)
            junk_c = one.tile([H, C], F32)
            nc.vector.tensor_mul(junk_c, cpT, cw1)
            nc.vector.tensor_reduce(pair[:, 0:1], junk_c,
                                    axis=mybir.AxisListType.X, op=ALU.max)

            ps_pw = ps_mm.tile([H, P], F32, tag="mmbig")
            for w in range(2):
                nc.tensor.matmul(ps_pw[:, w * W:(w + 1) * W], WiT_r,
                                 Sp_sb[:, w * W:(w + 1) * W], start=True, stop=True)
            pw1 = one.tile([H, P], F32)
            nc.scalar.activation(out=pw1, in_=ps_pw, func=AF.Sigmoid,
                                 bias=bi_col, scale=1.0 / C)
            nc.vector.tensor_scalar(out=pw1, in0=pw1, scalar1=0.5, scalar2=None,
                                    op0=ALU.add)
            junk_p = one.tile([H, P], F32)
            nc.vector.tensor_mul(junk_p, ppT.bitcast(F32), pw1)
            nc.vector.tensor_reduce(pair[:, 1:2], junk_p,
                                    axis=mybir.AxisListType.X, op=ALU.max)

            # final MLP (row form)
            W1T = one.tile([128, 2, 8, 128], F32R)
            for kj in range(2):
                for g in range(2):
                    transpose_pack(W1T[:, kj, 4 * g:4 * g + 4, :],
                                   [W1_nat[:, 4 * g + j, kj * 128:(kj + 1) * 128]
                                    for j in range(4)], F32R)
            W2T = one.tile([128, 8, 4, 128], F32R)
            for kj in range(8):
                transpose_pack(W2T[:, kj, :, :],
                               [W2_nat[:, j, kj * 128:(kj + 1) * 128]
                                for j in range(4)], F32R)

            pair_r = one.tile([H, 2], F32R)
            nc.vector.tensor_copy(pair_r, pair)

            ps_x1r = ps_mm.tile([1, 1024], F32, tag="mmbig")
            for w in range(2):
                for kj in range(2):
                    nc.tensor.matmul(ps_x1r[:, w * W:(w + 1) * W],
                                     pair_r[:, kj:kj + 1],
                                     W1T[:, kj, :, :].rearrange(
                                         "k a b -> k (a b)")[:, w * W:(w + 1) * W],
                                     start=(kj == 0), stop=(kj == 1))
            x1row = one.tile([1, 1024], F32)
            nc.scalar.copy(x1row, ps_x1r)
            x1 = one.tile([128, 8], F32)
            for mi in range(8):
                pst1 = ps_tp.tile([128, 512], F32, tag="tp")
                nc.tensor.matmul(pst1[:, 0:1], x1row[0:1, mi * 128:(mi + 1) * 128],
                                 ident_f[0:1, 0:1], is_transpose=True,
                                 start=True, stop=True)
                nc.scalar.activation(out=x1[:, mi:mi + 1], in_=pst1[:, 0:1],
                                     func=AF.Identity, bias=b1_col[:, mi:mi + 1],
                                     scale=1.0)
            x1s = one.tile([128, 8], F32)
            nc.vector.tensor_scalar(out=x1s, in0=x1, scalar1=0.01, scalar2=None,
                                    op0=ALU.mult)
            nc.vector.tensor_max(x1, x1, x1s)
            x1_r = one.tile([128, 8], F32R)
            nc.vector.tensor_copy(x1_r, x1)

            ps_x2r = ps_mm.tile([1, 512], F32, tag="mmbig")
            for kj in range(8):
                nc.tensor.matmul(ps_x2r, x1_r[:, kj:kj + 1],
                                 W2T[:, kj, :, :].rearrange("k a b -> k (a b)"),
                                 start=(kj == 0), stop=(kj == 7))
            x2row = one.tile([1, 512], F32)
            nc.scalar.copy(x2row, ps_x2r)
            x2 = one.tile([128, 4], F32)
            for mi in range(4):
                pst2 = ps_tp.tile([128, 512], F32, tag="tp")
                nc.tensor.matmul(pst2[:, 0:1], x2row[0:1, mi * 128:(mi + 1) * 128],
                                 ident_f[0:1, 0:1], is_transpose=True,
                                 start=True, stop=True)
                nc.scalar.activation(out=x2[:, mi:mi + 1], in_=pst2[:, 0:1],
                                     func=AF.Identity, bias=b2_col[:, mi:mi + 1],
                                     scale=1.0)
            x2s = one.tile([128, 4], F32)
            nc.vector.tensor_scalar(out=x2s, in0=x2, scalar1=0.01, scalar2=None,
                                    op0=ALU.mult)
            nc.vector.tensor_max(x2, x2, x2s)

            ps_o = ps_sm.tile([2, 1], F32, tag="small")
            for kj in range(4):
                nc.tensor.matmul(ps_o, WoT[:, kj, :], x2[:, kj:kj + 1],
                                 start=(kj == 0), stop=(kj == 3))
            out_sb = one.tile([2, 1], F32)
            nc.scalar.activation(out=out_sb, in_=ps_o, func=AF.Identity,
                                 bias=bo_col, scale=1.0)
            nc.sync.dma_start(out_d, out_sb)

    nc.compile()
    return nc


def kernel(**inputs):
    from concourse.bass_utils import run_bass_kernel_spmd

    nc = _CACHE.get("nc")
    if nc is None:
        nc = _build()
        _CACHE["nc"] = nc

    weight_names = ["Wp", "bp", "Wc", "bc", "Wpa", "bpa", "Wca", "bca",
                    "Wi", "bi", "W1", "b1", "W2", "b2", "Wo", "bo"]
    in_maps = []
    for b in range(B):
        m = {"protein": np.ascontiguousarray(inputs["protein"][b], dtype=np.float32),
             "compound": np.ascontiguousarray(inputs["compound"][b], dtype=np.float32)}
        for k in weight_names:
            m[k] = np.ascontiguousarray(inputs[k], dtype=np.float32)
        in_maps.append(m)

    res = run_bass_kernel_spmd(nc, in_maps, core_ids=list(range(NCORES)))
    out = np.stack([res.results[b]["out"].reshape(2) for b in range(B)])
    return out.astype(np.float32)
